# revision 37
# baseline (speedup 1.0000x reference)
"""Trainium2 Bass kernel for ExpertMLPLoRA (moe_routing).

Reference computation (per batch b, selected expert k):
    A = A_all[expert_indices]            # [K, D, R]
    Bm = B_all[expert_indices]           # [K, R, D]
    down = einsum('bkmd,kdr->bkmr', z, A)
    up   = einsum('bkmr,krd->bkmd', down, Bm)
    out  = up * (alpha/rank)

Sharding: data-parallel over batch B=8 -> one batch per NeuronCore.
Each core receives its z[b] slice plus the full (replicated) LoRA
tables and gathers the K=8 selected experts on-device via indirect
DMA.  Host only pre-expands the 8 expert indices into row indices
(pure address arithmetic).

Device pipeline per (b, k):
  1. SWDGE cast-DMA z[b,k] [512, 1024] f32 HBM -> bf16 SBUF [128p, (mc,d)]
  2. 32x PE transpose (bf16, via identity matmul) -> z^T chunks in PSUM,
     copied back to SBUF on DVE/ACT
  3. mm1: 8 singleton matmuls A_chunk[128d,16r].T @ z^T[128d,512m] ->
     PSUM partials, pair-added on ACT/DVE + GpSimd tree (PSUM
     accumulation groups fault this hw when foreign matmuls interleave)
  4. mm2: up[128m, 512d] = down^T_slice[16,128].T @ B_k[16,512]
  5. copy PSUM -> SBUF f32 out tile (DVE/ACT), DMA 2 MiB back to HBM

The LoRA scale folds into the bf16 cast of the gathered A table.
"""

import numpy as np

_B, _K, _M, _D, _R = 8, 8, 512, 1024, 16
_SCALE = 1.0 / _R
_NCORES = 8

_cache = {}


def _apply_tile_drain_patch():
    """This walrus build caps sync waits at 1 per instruction (2 for
    EventSemaphore).  Tile's kernel-tail drain piles every final sem wait
    onto one Drain -> NCC_INLA001 'Too many sync wait commands'.  Re-emit
    the extras as standalone per-sem waits before the drain."""
    import concourse.tile as tile_mod
    from concourse.tile import TileContext

    if getattr(TileContext, "_drain_patch_applied", False):
        return
    try:
        from concourse.tile import ScopedClock
    except ImportError:
        from bass_rust import ScopedClock

    def _patched(self, tick_clock, wait_clock):
        nc = self.nc
        probe = nc.sync.drain()
        wait_clock.add_sem_waits(
            probe.ins, ScopedClock({None: tick_clock.global_clock})
        )
        waits = list(probe.ins.sync_info.on_wait)
        if len(waits) > 1:
            assert self.sems is not None
            by_name = {s.name: s for s in self.sems.allocated().values()}
            for w in waits[1:]:
                sem = by_name.get(w.ant_name)
                assert sem is not None, f"semaphore {w.ant_name} not found"
                nc.sync.wait_ge(sem, w.wait_value)
            probe.ins.sync_info.on_wait = waits[:1]
            nc.sync.drain()
        nc.all_engine_barrier()
        assert self.sems is not None
        popped = nc._tile_sem_poison_stack.pop()
        assert popped is self._sem_poison
        nc.clear_and_free_semaphores(list(self.sems.allocated().values()))
        nc.all_engine_barrier()

    TileContext._drain_and_barrier = _patched
    TileContext._drain_patch_applied = True


def _split_excess_waits(nc):
    """This walrus build rejects instructions carrying more than 1-2 sync
    waits ('Too many sync wait commands'), but Tile's sem-assignment packs
    up to ~9 waits onto one instruction.  Hoist the excess onto standalone
    EventSemaphore carriers placed immediately before the instruction on
    the same engine (engines execute in order, so blocking semantics are
    identical)."""
    import bass_rust
    import concourse.mybir as mybir

    n = 0
    for fn in nc.m.functions:
        for bb in fn.blocks:
            new_insts = []
            for inst in bb.instructions:
                si = inst.sync_info
                waits = list(si.on_wait) if si is not None else []
                cap = 2 if isinstance(inst, mybir.InstEventSemaphore) else 1
                if len(waits) > cap:
                    for w in waits[cap:]:
                        n += 1
                        new_insts.append(
                            mybir.InstEventSemaphore(
                                name=f"wsplit-{n}-{inst.name}",
                                engine=inst.engine,
                                ins=[],
                                outs=[],
                                sync_info=bass_rust.SyncInfo(
                                    on_wait=[w], on_update=[]
                                ),
                            )
                        )
                    inst.sync_info = bass_rust.SyncInfo(
                        on_wait=waits[:cap], on_update=list(si.on_update)
                    )
                new_insts.append(inst)
            bb.instructions = new_insts
    return n


def _build(split_waits=True):
    import concourse.bass as bass
    import concourse.mybir as mybir
    from concourse.masks import make_identity
    from concourse.tile import TileContext

    _apply_tile_drain_patch()
    f32 = mybir.dt.float32
    bf16 = mybir.dt.bfloat16
    i32 = mybir.dt.int32

    nc = bass.Bass()
    z = nc.declare_dram_parameter("z", [_K, _M, _D], f32, isOutput=False)
    # A_all [64, 1024, 16] viewed as rows (e, dc) of [128, 16] blocks
    a_tab = nc.declare_dram_parameter("a_tab", [64 * 8, 128 * _R], f32, isOutput=False)
    # B_all [64, 16, 1024] viewed as rows (e, r) of [1024] d-vectors
    b_tab = nc.declare_dram_parameter("b_tab", [64 * _R, _D], f32, isOutput=False)
    idxa = nc.declare_dram_parameter("idxa", [64, 1], i32, isOutput=False)
    idxb = nc.declare_dram_parameter("idxb", [128, 1], i32, isOutput=False)
    out = nc.declare_dram_parameter("out", [_K, _M, _D], f32, isOutput=True)

    with TileContext(nc) as tc:
        with (
            tc.tile_pool(name="const", bufs=1) as cpool,
            tc.tile_pool(name="io", bufs=3) as iopool,
            tc.tile_pool(name="acc", bufs=2) as apool,
            tc.tile_pool(name="psd", bufs=2, space="PSUM") as psd,
            tc.tile_pool(name="psu", bufs=2, space="PSUM") as psu,
            tc.tile_pool(name="pst", bufs=2, space="PSUM") as pst,
        ):
            def load_zb(k):
                # SWDGE cast-DMA: f32 HBM -> bf16 SBUF in one transfer
                zb = iopool.tile([128, 4096], bf16, tag="zb")
                nc.gpsimd.dma_start(
                    out=zb[:].rearrange("p (mc d) -> p mc d", mc=4),
                    in_=z[k].rearrange("(mc p) d -> p mc d", p=128),
                )
                return zb

            ident = cpool.tile([128, 128], bf16)
            make_identity(nc, ident[:])

            # ---- one-time expert gather + layout prep ----
            ia = cpool.tile([64, 1], i32)
            nc.sync.dma_start(out=ia[:], in_=idxa[:])

            # gather A rows (k,dc) -> [64, 2048]; row content is [128p, 16r]
            a_rows = cpool.tile([64, 2048], f32)
            nc.gpsimd.indirect_dma_start(
                out=a_rows[:],
                out_offset=None,
                in_=a_tab[:],
                in_offset=bass.IndirectOffsetOnAxis(ap=ia[:, :1], axis=0),
            )
            # identity for PE transposes (f32 copy for the A-table prep)
            identf = cpool.tile([128, 128], f32)
            make_identity(nc, identf[:])
            # redistribute d across partitions with 16 strided PE
            # transposes (one per rank index r): [64(k,dc), 128(d)] -> psum
            # [128(d), (r, k, dc)], then one free-dim-permuted DVE copy with
            # the LoRA scale and bf16 cast folded in.
            a_rows_v = a_rows[:].rearrange("j (p r) -> j r p", r=_R)
            pa = psu.tile([128, 1024], f32, tag="up")
            for r in range(_R):
                nc.tensor.transpose(
                    out=pa[:, r * 64 : (r + 1) * 64],
                    in_=a_rows_v[:, r, :],
                    identity=identf[:64, :64],
                )
            a_tb = cpool.tile([128, 8 * 8 * _R], bf16)
            nc.vector.tensor_scalar_mul(
                a_tb[:].rearrange("p (k dc r) -> p r (k dc)", k=8, r=_R),
                pa[:].rearrange("p (r j) -> p r j", r=_R),
                _SCALE,
            )

            # start streaming z for k=0 before the B gathers occupy Q7
            zb0 = load_zb(0)

            # gather B rows (e,r) -> per-k [16r, 1024d] tiles (matmul
            # operands must sit at SBUF base partition 0)
            b_kt = []
            for k in range(_K):
                ibk = cpool.tile([16, 1], i32, tag=f"ibk{k}")
                nc.sync.dma_start(out=ibk[:], in_=idxb[k * 16 : (k + 1) * 16, :])
                bt = cpool.tile([16, _D], f32, tag=f"braw{k}")
                nc.gpsimd.indirect_dma_start(
                    out=bt[:],
                    out_offset=None,
                    in_=b_tab[:],
                    in_offset=bass.IndirectOffsetOnAxis(ap=ibk[:, :1], axis=0),
                )
                btb = cpool.tile([16, _D], bf16, tag=f"bb{k}")
                nc.vector.tensor_copy(out=btb[:], in_=bt[:])
                b_kt.append(btb)

            # ---- main loop over the K selected experts ----
            for k in range(_K):
                zb = zb0 if k == 0 else load_zb(k)

                # z^T chunks via PE transpose (bf16, 1 cycle/row):
                #   zt[p, dc*512+m] = z[k, m, dc*128+p]
                zt = iopool.tile([128, 4096], bf16, tag="zt")
                for dh in range(4):
                    pt = pst.tile([128, 1024], bf16, tag="zt_ps")
                    for dj in range(2):
                        dc = dh * 2 + dj
                        for mc in range(4):
                            nc.tensor.transpose(
                                out=pt[:, dj * 512 + mc * 128 : dj * 512 + (mc + 1) * 128],
                                in_=zb[:, mc * 1024 + dc * 128 : mc * 1024 + (dc + 1) * 128],
                                identity=ident[:],
                            )
                    dst = zt[:, dh * 1024 : (dh + 1) * 1024]
                    if dh % 2 == 0:
                        nc.vector.tensor_copy(out=dst, in_=pt[:])
                    else:
                        nc.scalar.copy(out=dst, in_=pt[:])

                # mm1: down^T [16, 512] = sum of 8 singleton-matmul partials.
                # (PSUM accumulation groups fault this hardware when foreign
                # matmuls interleave, so reduce outside the PE: pair-add the
                # PSUM partials on the DVE, finish the tree on idle GpSimd.)
                t4 = []
                for dc in range(8):
                    pd = psd.tile([16, 512], f32, tag="down")
                    nc.tensor.matmul(
                        out=pd[:],
                        lhsT=a_tb[:, (k * 8 + dc) * _R : (k * 8 + dc + 1) * _R],
                        rhs=zt[:, dc * 512 : (dc + 1) * 512],
                        start=True,
                        stop=True,
                    )
                    if dc % 2 == 0:
                        t = apool.tile([16, 512], f32, tag=f"t{dc // 2}")
                        nc.scalar.copy(out=t[:], in_=pd[:])
                        t4.append(t)
                    else:
                        t = t4[dc // 2]
                        nc.vector.tensor_add(out=t[:], in0=t[:], in1=pd[:])
                u0 = apool.tile([16, 512], f32, tag="u0")
                nc.gpsimd.tensor_add(out=u0[:], in0=t4[0][:], in1=t4[1][:])
                u1 = apool.tile([16, 512], f32, tag="u1")
                nc.gpsimd.tensor_add(out=u1[:], in0=t4[2][:], in1=t4[3][:])
                db = apool.tile([16, 512], bf16, tag="db")
                nc.vector.tensor_add(out=db[:], in0=u0[:], in1=u1[:])

                # mm2 + copy out
                ov = iopool.tile([128, 4096], f32, tag="ov")
                for mc2 in range(4):
                    pu = psu.tile([128, 1024], f32, tag="up")
                    for dc2 in range(2):
                        nc.tensor.matmul(
                            out=pu[:, dc2 * 512 : (dc2 + 1) * 512],
                            lhsT=db[:, mc2 * 128 : (mc2 + 1) * 128],
                            rhs=b_kt[k][:, dc2 * 512 : (dc2 + 1) * 512],
                            start=True,
                            stop=True,
                        )
                    dst = ov[:, mc2 * 1024 : (mc2 + 1) * 1024]
                    if mc2 % 2 == 0:
                        nc.vector.tensor_copy(out=dst, in_=pu[:])
                    else:
                        nc.scalar.copy(out=dst, in_=pu[:])
                nc.sync.dma_start(
                    out=out[k].rearrange("(mc p) d -> p mc d", p=128),
                    in_=ov[:].rearrange("p (mc d) -> p mc d", mc=4),
                )
    if split_waits:
        _split_excess_waits(nc)
    return nc


def kernel(z, A_all, B_all, expert_indices, _trace=False):
    from concourse.bass_utils import run_bass_kernel_spmd

    z = np.ascontiguousarray(np.asarray(z, dtype=np.float32))
    A_all = np.ascontiguousarray(np.asarray(A_all, dtype=np.float32))
    B_all = np.ascontiguousarray(np.asarray(B_all, dtype=np.float32))
    idx = np.asarray(expert_indices).astype(np.int64)
    assert z.shape == (_B, _K, _M, _D)

    if "nc" not in _cache:
        _cache["nc"] = _build()
    nc = _cache["nc"]

    a_tab = A_all.reshape(64 * 8, 128 * _R)
    b_tab = B_all.reshape(64 * _R, _D)
    idxa = (idx[:, None] * 8 + np.arange(8)[None, :]).reshape(64, 1).astype(np.int32)
    idxb = (idx[:, None] * 16 + np.arange(16)[None, :]).reshape(128, 1).astype(np.int32)

    in_maps = [
        {"z": z[c], "a_tab": a_tab, "b_tab": b_tab, "idxa": idxa, "idxb": idxb}
        for c in range(_NCORES)
    ]
    res = run_bass_kernel_spmd(nc, in_maps, list(range(_NCORES)), trace=_trace)
    globals()["last_exec_time_ns"] = res.exec_time_ns
    return np.stack([res.results[c]["out"] for c in range(_NCORES)], axis=0)
